# revision 26
# baseline (speedup 1.0000x reference)
"""Sliding-window causal self-attention (GQA + RoPE + QK-RMSNorm + ve-gate) on
8 Trainium2 NeuronCores.

Sharding: core c handles (batch b = c // 4, kv-head g = c % 4): data parallel
over batch x tensor parallel over the 4 KV head groups (4 query heads per
core). Each core computes its partial c_proj output; the all-reduce over the 4
head shards is a host-side sum.

v3 design (per core):
  - everything the PE touches is bf16 (inputs are host-converted); PSUM
    accumulation stays fp32, so matmul error is input-quantization only.
  - the ve gate (3*sigmoid(x[:,:12] @ Wgate)) is folded into ve on the host:
    ve' = gate * ve, so the device only does v += ve'.
  - k's rms-norm is folded into kT at PSUM evacuation (broadcast row * PSUM),
    so exp() needs no per-key scale and there are no DRAM round trips.
  - v is computed directly in natural (token-partition) layout by using the
    x chunk as the matmul stationary operand: no PE transposes. The k and v
    chunk loops are fused so slice-0 projections track the x DMA arrivals.
  - RoPE's half-swap uses DVE reads at a shifted partition base (the ss table
    is laid out [-sin; sin] so both SBUF inputs of each TensorTensor share a
    partition base, which the BIR verifier requires).
  - scores are computed transposed (S^T: tk x tq); softmax denominators come
    from a ones-stationary matmul into a shared [97, TS] PSUM tile (rows at
    32h: matmul outputs must start at partition 0/32/64/96); no
    max-subtraction (QK rms-norm bounds |score| <= 1.44*sqrt(128)); masking
    multiplies boundary tiles by 0/1 triangles on the Pool engine.
  - c_proj of slice m-1 is interleaved as single-matmul fillers between the
    attention tiles of slice m: the in-order PE would otherwise park at
    sum(i) waiting for exp(i) on the ACT engine (ACT is 2x slower per column
    than the PE).
  - q-head projections alternate between two PSUM pools so the
    square->rownorm->broadcast->evac chain of head h never blocks head h+1.
  - DMA count is ~41 (vs 251 in the original): weights/tables are
    host-prepacked into SBUF layout ([128, free]) so each is one
    large-descriptor DMA; x streams in 4 group-DMAs per 512-token slice;
    output streams out in 4 group-DMAs per slice (bf16 partials, host sums
    in fp32).
"""

import sys

sys.path.insert(0, "/opt/trn_rl_repo")

import numpy as np

B, T, C = 2, 2048, 2048
NH, NKV, HD = 16, 4, 128
GATE_CH = 12
HPC = NH // NKV          # q heads per core
TS = 512                 # token-slice width
NSL = T // TS            # 4 slices
NCK = C // 128           # 16 contraction chunks
TPS = TS // 128          # 4 token tiles per slice
NTT = T // 128           # 16 token tiles
EPS = 1e-6

A_Q = 1.2 / np.sqrt(float(HD))   # rms-norm scale folded into q (incl 1/sqrt(HD))
A_K = 1.2                        # rms-norm scale folded into k
S_Q = float(1.0 / (HD * A_Q * A_Q))
B_Q = float(EPS / (A_Q * A_Q))
S_K = float(1.0 / (HD * A_K * A_K))
B_K = float(EPS / (A_K * A_K))

_compiled = {}


def _ktiles(m4, W):
    """k-tiles overlapping q-slice m4 with their valid tq-column extents.

    Returns list of (n, f0, f1, causal_block_col, edge_block_col); columns are
    relative to the slice (0..TS). First entry covers [0, TS) fully (it opens
    the PSUM accumulation group).
    """
    assert W % 128 == 0 and W >= 384
    out = []
    for n in range(0, TPS * m4 + TPS):
        f0 = max(0, 128 * n - TS * m4)
        f1 = min(TS, 128 * n + W + 128 - TS * m4)
        if f1 <= f0:
            continue
        causal = 128 * n >= TS * m4            # diagonal staircase inside tile
        edge = (128 * n + W + 128 - TS * m4) <= TS  # window lower edge inside
        cb = f0 if causal else None
        eb = (f1 - 128) if edge else None
        out.append((n, f0, f1, cb, eb))
    full = [e for e in out if e[1] == 0 and e[2] == TS]
    assert full, "need one full-extent tile to open the PSUM group"
    first = full[0]
    rest = [e for e in out if e[0] != first[0]]
    return [first] + rest


def _build(W):
    import concourse.bass as bass
    import concourse.tile as tile
    from concourse import bacc, mybir
    from contextlib import ExitStack

    f32 = mybir.dt.float32
    bf16 = mybir.dt.bfloat16
    AF = mybir.ActivationFunctionType

    nc = bacc.Bacc(None, target_bir_lowering=False)

    xd = nc.dram_tensor("xp", [128, NCK, T], bf16, kind="ExternalInput")
    wqd = nc.dram_tensor("wqp", [128, NCK * HPC * HD], bf16, kind="ExternalInput")
    wkd = nc.dram_tensor("wkp", [128, NCK * HD], bf16, kind="ExternalInput")
    wvd = nc.dram_tensor("wvp", [128, NCK * HD], bf16, kind="ExternalInput")
    wpd = nc.dram_tensor("wpp", [128, HPC * C], bf16, kind="ExternalInput")
    ccd = nc.dram_tensor("cc", [128, T], bf16, kind="ExternalInput")
    ssd = nc.dram_tensor("ss", [128, T], bf16, kind="ExternalInput")
    ved = nc.dram_tensor("vep", [128, NTT * HD], bf16, kind="ExternalInput")
    trid = nc.dram_tensor("tri", [128, 384], bf16, kind="ExternalInput")
    outd = nc.dram_tensor("outp", [128, NTT, T], bf16, kind="ExternalOutput")

    with tile.TileContext(nc) as tc, ExitStack() as ctx:
        res = ctx.enter_context(tc.tile_pool(name="res", bufs=1))
        xc_p = ctx.enter_context(tc.tile_pool(name="xc", bufs=2))
        sq_p = ctx.enter_context(tc.tile_pool(name="sq", bufs=2))
        row_p = ctx.enter_context(tc.tile_pool(name="rows", bufs=2))
        bc_p = ctx.enter_context(tc.tile_pool(name="bc", bufs=3))
        qt_p = ctx.enter_context(tc.tile_pool(name="qt", bufs=2))
        es_p = ctx.enter_context(tc.tile_pool(name="es", bufs=4))
        yt_p = ctx.enter_context(tc.tile_pool(name="yt", bufs=2))
        work_p = ctx.enter_context(tc.tile_pool(name="work", bufs=2))
        ot_p = ctx.enter_context(tc.tile_pool(name="ot", bufs=3))

        # PSUM: 8 banks total. qkv(2: k then v, then cproj co rotation)
        # + s(3: q h0/h2, then the attention S pipeline) + out(2: q h1/h3,
        # then attention ps_out rotation) + misc(1: one bank shared serially
        # by the rms row sums (row 0) and the softmax denominators (rows
        # 0/32/64; matmul out bases are limited to 0/32/64)).
        ps_qkv = ctx.enter_context(tc.tile_pool(name="ps_qkv", bufs=2, space="PSUM"))
        ps_s = ctx.enter_context(tc.tile_pool(name="ps_s", bufs=3, space="PSUM"))
        ps_out_p = ctx.enter_context(tc.tile_pool(name="ps_out", bufs=2, space="PSUM"))
        ps_misc = ctx.enter_context(tc.tile_pool(name="ps_misc", bufs=1, space="PSUM"))

        # ---- resident tensors ----
        wq_sb = res.tile([128, NCK, HPC * HD], bf16)
        wk_sb = res.tile([128, NCK, HD], bf16)
        wv_sb = res.tile([128, NCK, HD], bf16)
        wp_sb = res.tile([128, HPC, C], bf16)
        cc_sb = res.tile([128, T], bf16)
        ss_sb = res.tile([128, T], bf16)
        ve_sb = res.tile([128, NTT, HD], bf16)
        tri_sb = res.tile([128, 384], bf16)  # [Mc^T | Me^T | identity]
        kT_sb = res.tile([128, T], bf16)        # rotated+normalized k, HD on partitions
        vn_sb = res.tile([128, NTT, HD], bf16)  # v natural, token tiles on partitions
        ones_sb = res.tile([128, 1], bf16)
        nc.vector.memset(ones_sb, 1.0)
        bq_sb = res.tile([1, 1], f32)
        nc.vector.memset(bq_sb, B_Q)
        bk_sb = res.tile([1, 1], f32)
        nc.vector.memset(bk_sb, B_K)

        xcs = {}
        auxs = {}

        def emit_xload(m4):
            xc = xc_p.tile([128, NCK, TS], bf16, tag="xc")
            t0 = m4 * TS
            for g in range(4):
                nc.sync.dma_start(out=xc[:, 4 * g:4 * g + 4, :],
                                  in_=xd[:, 4 * g:4 * g + 4, t0:t0 + TS])
            xcs[m4] = xc

        def rope(dst, t0):
            """In-place RoPE on a [128, TS] bf16 tile; halves swap via DVE
            reads at a shifted partition base. ss rows 64:128 hold sin, rows
            0:64 hold -sin, matching each op's shared input base."""
            tmp = work_p.tile([128, TS], bf16, tag="tmp")
            nc.vector.tensor_mul(tmp[0:64, :], dst[64:128, :], ss_sb[64:128, t0:t0 + TS])
            nc.vector.tensor_mul(tmp[64:128, :], dst[0:64, :], ss_sb[0:64, t0:t0 + TS])
            nc.vector.tensor_mul(dst, dst, cc_sb[:, t0:t0 + TS])
            nc.vector.tensor_add(dst, dst, tmp)

        def emit_kvproj(m4):
            t0 = m4 * TS
            xc = xcs[m4]
            # k chunk loop, then v in tt-major order: each start=True marks the
            # whole 2048B zero region (= full bank row) pending-zero, so the
            # four v accumulation regions sharing one bank must run strictly
            # one after another — interleaving them clobbers siblings.
            ps_k = ps_qkv.tile([128, TS], f32, tag="qkv")
            for c in range(NCK):
                nc.tensor.matmul(ps_k, wk_sb[:, c, :], xc[:, c, :],
                                 start=(c == 0), stop=(c == NCK - 1))
            ps_v = ps_qkv.tile([128, TPS, HD], f32, tag="qkv")
            for tt in range(TPS):
                for c in range(NCK):
                    nc.tensor.matmul(ps_v[:, tt, :], xc[:, c, tt * 128:(tt + 1) * 128],
                                     wv_sb[:, c, :], start=(c == 0), stop=(c == NCK - 1))
            aux = ps_misc.tile([65, TS], f32, tag="misc")
            auxs[m4] = aux
            sq_k = sq_p.tile([128, TS], bf16, tag="sq")
            nc.scalar.activation(sq_k, ps_k, AF.Square)
            nc.tensor.matmul(aux[0:1, :], ones_sb, sq_k, start=True, stop=True)
            srk = row_p.tile([1, TS], f32, tag="srk")
            nc.scalar.activation(srk, aux[0:1, :], AF.Ln, bias=bk_sb, scale=S_K)
            nc.scalar.activation(srk, srk, AF.Exp, scale=-0.5)
            rbk = bc_p.tile([128, TS], f32, tag="bc")
            nc.gpsimd.partition_broadcast(rbk, srk)
            k_sl = kT_sb[:, t0:t0 + TS]
            nc.vector.tensor_mul(k_sl, ps_k, rbk)
            rope(k_sl, t0)
            for tt in range(TPS):
                nc.vector.tensor_add(vn_sb[:, m4 * TPS + tt, :], ps_v[:, tt, :],
                                     ve_sb[:, m4 * TPS + tt, :])

        def emit_qproj(m4):
            t0 = m4 * TS
            xc = xcs[m4]
            aux = auxs[m4]
            qts = []
            for h in range(HPC):
                pool = ps_s if h % 2 == 0 else ps_out_p
                tag = "s" if h % 2 == 0 else "out"
                ps_q = pool.tile([128, TS], f32, tag=tag)
                for c in range(NCK):
                    nc.tensor.matmul(ps_q, wq_sb[:, c, h * HD:(h + 1) * HD], xc[:, c, :],
                                     start=(c == 0), stop=(c == NCK - 1))
                sq_q = sq_p.tile([128, TS], bf16, tag="sq")
                nc.scalar.activation(sq_q, ps_q, AF.Square)
                # the single aux row is serially reused by k and all q heads:
                # each row is consumed by the Ln activation ~1.2us after it is
                # written, long before the next head's ones-matmul lands.
                nc.tensor.matmul(aux[0:1, :], ones_sb, sq_q, start=True, stop=True)
                srow = row_p.tile([1, TS], f32, tag="srow")
                nc.scalar.activation(srow, aux[0:1, :], AF.Ln,
                                     bias=bq_sb, scale=S_Q)
                nc.scalar.activation(srow, srow, AF.Exp, scale=-0.5)
                rbc = bc_p.tile([128, TS], f32, tag="bc")
                nc.gpsimd.partition_broadcast(rbc, srow)
                qt = qt_p.tile([128, TS], bf16, tag=f"qt{h}")
                nc.vector.tensor_mul(qt, ps_q, rbc)
                rope(qt, t0)
                qts.append(qt)
            return qts

        def make_cproj_fillers(m4, yts, tail=False):
            """One thunk per c_proj matmul of slice m4 (64 total), in co-major
            order; each accumulates into a rotating ps_qkv bank, evacuates
            at h==3, and DMAs out each finished group of 4 co's. For the
            kernel tail the last group streams out per-co so the final DMA
            only carries 128KB."""
            t0 = m4 * TS
            state = {}
            fillers = []

            def make(co, h):
                def f():
                    percol = tail and co >= NTT - 4
                    if h == 0 and co % 4 == 0:
                        state["ot"] = ot_p.tile([128, 4, TS], bf16, tag="ot",
                                                name="ot")
                    if h == 0:
                        state[co] = ps_qkv.tile([128, TS], f32, tag="qkv",
                                                name="ps_p")
                    nc.tensor.matmul(state[co],
                                     wp_sb[:, h, co * 128:(co + 1) * 128],
                                     yts[h], start=(h == 0), stop=(h == HPC - 1))
                    if h == HPC - 1:
                        nc.vector.tensor_copy(state["ot"][:, co % 4, :], state[co])
                        if percol:
                            nc.sync.dma_start(
                                out=outd[:, co:co + 1, t0:t0 + TS],
                                in_=state["ot"][:, co % 4:co % 4 + 1, :])
                        elif co % 4 == 3:
                            cg = co // 4
                            nc.sync.dma_start(
                                out=outd[:, 4 * cg:4 * cg + 4, t0:t0 + TS],
                                in_=state["ot"])
                return f

            if tail:
                # interleave the first two co's h-loops across the two banks
                # so the PE has six matmuls in hand before it needs yt3 (the
                # recip->broadcast->mul chain of the last head).
                for h in range(HPC):
                    fillers.append(make(0, h))
                    fillers.append(make(1, h))
                start_co = 2
            else:
                start_co = 0
            for co in range(start_co, NTT):
                for h in range(HPC):
                    fillers.append(make(co, h))
            return fillers

        def emit_attn(m4, qts, fillers):
            tiles = _ktiles(m4, W)
            last = len(tiles) - 1
            wts = [3 if i == 0 else (2 if i == 1 else 1) for i in range(len(tiles))]
            wsum = sum(wts) * HPC
            nfill = len(fillers)
            fi = 0
            slot = 0
            # denominators live in the same bank as the rms rows: heads 0-2
            # get rows 0/32/64, head 3 reuses row 0 (h0's recip read happened
            # two head-periods earlier).
            sum4 = auxs[m4]
            sum_row = [0, 32, 64, 0]
            yts = []
            for h in range(HPC):
                sr = sum_row[h]
                ps_out = ps_out_p.tile([128, TS], f32, tag="out")
                for idx, (n, f0, f1, cb, eb) in enumerate(tiles):
                    pss = ps_s.tile([128, TS], f32, tag="s")
                    masked = (cb is not None) + (eb is not None)
                    nc.tensor.matmul(pss[:, f0:f1], kT_sb[:, n * 128:(n + 1) * 128],
                                     qts[h][:, f0:f1], start=True, stop=(masked == 0))
                    # boundary masking as a pre-exp -100 additive matmul
                    # (mask^T stationary, identity moving) in the same PSUM
                    # group: exp then yields exact zeros and nothing waits on
                    # the Pool engine.
                    if cb is not None:
                        masked -= 1
                        nc.tensor.matmul(pss[:, cb:cb + 128], tri_sb[:, 0:128],
                                         tri_sb[:, 256:384], start=False,
                                         stop=(masked == 0))
                    if eb is not None:
                        masked -= 1
                        nc.tensor.matmul(pss[:, eb:eb + 128], tri_sb[:, 128:256],
                                         tri_sb[:, 256:384], start=False,
                                         stop=(masked == 0))
                    # place c_proj filler matmuls right after the S matmul:
                    # the PE chews these while the ACT engine runs exp(i).
                    # Pacing is weighted 3/2/1 toward the first tiles of each
                    # head — the exp pipeline needs runway there.
                    slot += wts[idx]
                    while fi < min(nfill, nfill * slot // wsum):
                        fillers[fi]()
                        fi += 1
                    es = es_p.tile([128, TS], bf16, tag="es")
                    nc.scalar.activation(es[:, f0:f1], pss[:, f0:f1], AF.Exp)
                    nc.tensor.matmul(sum4[sr:sr + 1, f0:f1], ones_sb,
                                     es[:, f0:f1], start=(idx == 0), stop=(idx == last))
                    nc.tensor.matmul(ps_out[:, f0:f1], vn_sb[:, n, :], es[:, f0:f1],
                                     start=(idx == 0), stop=(idx == last))
                rsum = row_p.tile([1, TS], f32, tag="rsum")
                nc.vector.reciprocal(rsum, sum4[sr:sr + 1, :])
                sbc = bc_p.tile([128, TS], f32, tag="bc")
                nc.gpsimd.partition_broadcast(sbc, rsum)
                yt = yt_p.tile([128, TS], bf16, tag=f"yt{h}")
                nc.vector.tensor_mul(yt, ps_out, sbc)
                yts.append(yt)
            while fi < nfill:
                fillers[fi]()
                fi += 1
            return yts

        # ---- prologue ----
        # x groups stream on the SP HWDGE queue; all other input loads go on
        # the Activation HWDGE queue so their fixed per-DMA overheads overlap
        # the x stream (transfers still serialize on the DMA engines, but the
        # issue pipeline does not).
        emit_xload(0)
        nc.scalar.dma_start(out=wk_sb, in_=wkd[:, :].rearrange("p (c h) -> p c h", c=NCK))
        nc.scalar.dma_start(out=wv_sb, in_=wvd[:, :].rearrange("p (c h) -> p c h", c=NCK))
        nc.scalar.dma_start(out=wq_sb, in_=wqd[:, :].rearrange("p (c h) -> p c h", c=NCK))
        nc.scalar.dma_start(out=cc_sb, in_=ccd[:, :])
        nc.scalar.dma_start(out=ss_sb, in_=ssd[:, :])
        nc.scalar.dma_start(out=ve_sb, in_=ved[:, :].rearrange("p (n h) -> p n h", n=NTT))
        nc.scalar.dma_start(out=tri_sb, in_=trid[:, :])
        nc.scalar.dma_start(out=wp_sb, in_=wpd[:, :].rearrange("p (g o) -> p g o", g=HPC))

        # ---- software-pipelined slice loop ----
        emit_kvproj(0)
        prev = None
        for m4 in range(NSL):
            if m4 + 1 < NSL:
                emit_xload(m4 + 1)
            qts = emit_qproj(m4)
            fillers = make_cproj_fillers(*prev) if prev else []
            yts = emit_attn(m4, qts, fillers)
            if m4 + 1 < NSL:
                emit_kvproj(m4 + 1)
            prev = (m4, yts)
        for f in make_cproj_fillers(*prev, tail=True):
            f()

    # Restrict the activation-table picker to the one set containing every
    # ACT function we use (exp, ln, square, copy, identity): without this the
    # greedy picker alternates exp_and_others <-> natural_log, inserting a
    # ~1.3us table load per switch. Set ids are positions in act_info.json's
    # list, so unwanted sets are emptied rather than removed.
    import concourse.hw_specs as hw_specs
    import concourse.bacc as bacc_mod

    orig = hw_specs.get_activation_tables

    def only_combined(arch):
        t = orig(arch)
        return {k: (v if k == "natural_log_exp_and_others" else set())
                for k, v in t.items()}

    hw_specs.get_activation_tables = only_combined
    bacc_mod.get_activation_tables = only_combined
    try:
        nc.compile()
    finally:
        hw_specs.get_activation_tables = orig
        bacc_mod.get_activation_tables = orig
    return nc


def _pack_rows(a):
    """[C_rows, F] -> [128, (C_rows/128) * F] SBUF layout (partition-major)."""
    from ml_dtypes import bfloat16

    rows, f = a.shape
    nck = rows // 128
    return np.ascontiguousarray(
        a.reshape(nck, 128, f).transpose(1, 0, 2).reshape(128, nck * f)
    ).astype(bfloat16)


def _prep_inputs(x, ve, cos, sin, Wq, Wk, Wv, Wproj, Wgate, W):
    from ml_dtypes import bfloat16

    cosT = cos[0, :, 0, :].T  # (64, T)
    sinT = sin[0, :, 0, :].T
    cc = np.concatenate([cosT, cosT], axis=0).astype(bfloat16)
    # rows 0:64 = -sin (used by tmp[64:128] = x1 * -sin, both operands base 0),
    # rows 64:128 = sin (used by tmp[0:64] = x2 * sin, both operands base 64).
    ss = np.concatenate([-sinT, sinT], axis=0).astype(bfloat16)
    # Additive -100 masks, pre-transposed for use as the matmul stationary
    # (out[p,f] += Mx^T[f,p] via an identity moving operand):
    #   causal block valid iff p <= f; edge block valid iff f <= p + W%128.
    p = np.arange(128)[:, None]
    f = np.arange(128)[None, :]
    mc = np.where(p <= f, 0.0, -100.0).astype(np.float32).T
    me = np.where(f <= p + (W % 128), 0.0, -100.0).astype(np.float32).T
    ident = np.eye(128, dtype=np.float32)
    tri = np.concatenate([mc, me, ident], axis=1).astype(bfloat16)

    xp_by_b = {}
    for b in range(B):
        xp_by_b[b] = _pack_rows(x[b].T)  # [128, NCK*T]

    in_maps = []
    for core in range(8):
        b, g = core // NKV, core % NKV
        hs = slice(g * HPC * HD, (g + 1) * HPC * HD)
        ks = slice(g * HD, (g + 1) * HD)
        gate = 3.0 / (1.0 + np.exp(-(x[b][:, :GATE_CH] @ Wgate[g])))  # (T,)
        vep = gate[:, None] * ve[b][:, ks]  # (T, HD)
        in_maps.append({
            "xp": xp_by_b[b],
            "wqp": _pack_rows(Wq[hs, :].T),
            "wkp": _pack_rows(Wk[ks, :].T),
            "wvp": _pack_rows(Wv[ks, :].T),
            "wpp": _pack_rows(Wproj[:, hs].T),
            "cc": cc,
            "ss": ss,
            "vep": _pack_rows(vep),
            "tri": tri,
        })
    return in_maps


def _run(inputs, trace=False):
    from concourse.bass_utils import run_bass_kernel_spmd

    x = np.asarray(inputs["x"], dtype=np.float32)
    ve = np.asarray(inputs["ve"], dtype=np.float32)
    cos = np.asarray(inputs["cos"], dtype=np.float32)
    sin = np.asarray(inputs["sin"], dtype=np.float32)
    Wq = np.asarray(inputs["Wq"], dtype=np.float32)
    Wk = np.asarray(inputs["Wk"], dtype=np.float32)
    Wv = np.asarray(inputs["Wv"], dtype=np.float32)
    Wproj = np.asarray(inputs["Wproj"], dtype=np.float32)
    Wgate = np.asarray(inputs["Wgate"], dtype=np.float32)
    W = int(inputs["window_size"])

    if W not in _compiled:
        _compiled[W] = _build(W)
    nc = _compiled[W]

    in_maps = _prep_inputs(x, ve, cos, sin, Wq, Wk, Wv, Wproj, Wgate, W)
    res = run_bass_kernel_spmd(nc, in_maps, core_ids=list(range(8)), trace=trace)

    out = np.zeros((B, T, C), dtype=np.float32)
    for core in range(8):
        b = core // NKV
        # outp: [128, NTT, T] with (p, co, t) = partial[co*128+p, t]
        part = np.asarray(res.results[core]["outp"]).astype(np.float32)
        out[b] += part.transpose(1, 0, 2).reshape(C, T).T
    return out, res


def kernel(**inputs):
    out, _ = _run(inputs, trace=False)
    return out


# revision 27
# speedup vs baseline: 1.0329x; 1.0329x over previous
"""Sliding-window causal self-attention (GQA + RoPE + QK-RMSNorm + ve-gate) on
8 Trainium2 NeuronCores.

Sharding: core c handles (batch b = c // 4, kv-head g = c % 4): data parallel
over batch x tensor parallel over the 4 KV head groups (4 query heads per
core). Each core computes its partial c_proj output; the all-reduce over the 4
head shards is a host-side sum.

v3 design (per core):
  - everything the PE touches is bf16 (inputs are host-converted); PSUM
    accumulation stays fp32, so matmul error is input-quantization only.
  - the ve gate (3*sigmoid(x[:,:12] @ Wgate)) is folded into ve on the host:
    ve' = gate * ve, so the device only does v += ve'.
  - k's rms-norm is folded into kT at PSUM evacuation (broadcast row * PSUM),
    so exp() needs no per-key scale and there are no DRAM round trips.
  - v is computed directly in natural (token-partition) layout by using the
    x chunk as the matmul stationary operand: no PE transposes. The k and v
    chunk loops are fused so slice-0 projections track the x DMA arrivals.
  - RoPE's half-swap uses DVE reads at a shifted partition base (the ss table
    is laid out [-sin; sin] so both SBUF inputs of each TensorTensor share a
    partition base, which the BIR verifier requires).
  - scores are computed transposed (S^T: tk x tq); softmax denominators come
    from a ones-stationary matmul into a shared [97, TS] PSUM tile (rows at
    32h: matmul outputs must start at partition 0/32/64/96); no
    max-subtraction (QK rms-norm bounds |score| <= 1.44*sqrt(128)); masking
    multiplies boundary tiles by 0/1 triangles on the Pool engine.
  - c_proj of slice m-1 is interleaved as single-matmul fillers between the
    attention tiles of slice m: the in-order PE would otherwise park at
    sum(i) waiting for exp(i) on the ACT engine (ACT is 2x slower per column
    than the PE).
  - q-head projections alternate between two PSUM pools so the
    square->rownorm->broadcast->evac chain of head h never blocks head h+1.
  - DMA count is ~41 (vs 251 in the original): weights/tables are
    host-prepacked into SBUF layout ([128, free]) so each is one
    large-descriptor DMA; x streams in 4 group-DMAs per 512-token slice;
    output streams out in 4 group-DMAs per slice (bf16 partials, host sums
    in fp32).
"""

import sys

sys.path.insert(0, "/opt/trn_rl_repo")

import numpy as np

B, T, C = 2, 2048, 2048
NH, NKV, HD = 16, 4, 128
GATE_CH = 12
HPC = NH // NKV          # q heads per core
TS = 512                 # token-slice width
NSL = T // TS            # 4 slices
NCK = C // 128           # 16 contraction chunks
TPS = TS // 128          # 4 token tiles per slice
NTT = T // 128           # 16 token tiles
EPS = 1e-6

A_Q = 1.2 / np.sqrt(float(HD))   # rms-norm scale folded into q (incl 1/sqrt(HD))
A_K = 1.2                        # rms-norm scale folded into k
S_Q = float(1.0 / (HD * A_Q * A_Q))
B_Q = float(EPS / (A_Q * A_Q))
S_K = float(1.0 / (HD * A_K * A_K))
B_K = float(EPS / (A_K * A_K))

_compiled = {}


def _ktiles(m4, W):
    """k-tiles overlapping q-slice m4 with their valid tq-column extents.

    Returns list of (n, f0, f1, causal_block_col, edge_block_col); columns are
    relative to the slice (0..TS). First entry covers [0, TS) fully (it opens
    the PSUM accumulation group).
    """
    assert W % 128 == 0 and W >= 384
    out = []
    for n in range(0, TPS * m4 + TPS):
        f0 = max(0, 128 * n - TS * m4)
        f1 = min(TS, 128 * n + W + 128 - TS * m4)
        if f1 <= f0:
            continue
        causal = 128 * n >= TS * m4            # diagonal staircase inside tile
        edge = (128 * n + W + 128 - TS * m4) <= TS  # window lower edge inside
        cb = f0 if causal else None
        eb = (f1 - 128) if edge else None
        out.append((n, f0, f1, cb, eb))
    full = [e for e in out if e[1] == 0 and e[2] == TS]
    assert full, "need one full-extent tile to open the PSUM group"
    first = full[0]
    rest = [e for e in out if e[0] != first[0]]
    return [first] + rest


def _build(W):
    import concourse.bass as bass
    import concourse.tile as tile
    from concourse import bacc, mybir
    from contextlib import ExitStack

    f32 = mybir.dt.float32
    bf16 = mybir.dt.bfloat16
    AF = mybir.ActivationFunctionType

    nc = bacc.Bacc(None, target_bir_lowering=False)

    xd = nc.dram_tensor("xp", [128, NCK, T], bf16, kind="ExternalInput")
    wqd = nc.dram_tensor("wqp", [128, NCK * HPC * HD], bf16, kind="ExternalInput")
    wkd = nc.dram_tensor("wkp", [128, NCK * HD], bf16, kind="ExternalInput")
    wvd = nc.dram_tensor("wvp", [128, NCK * HD], bf16, kind="ExternalInput")
    wpd = nc.dram_tensor("wpp", [128, HPC * C], bf16, kind="ExternalInput")
    ccd = nc.dram_tensor("cc", [128, T], bf16, kind="ExternalInput")
    ssd = nc.dram_tensor("ss", [128, T], bf16, kind="ExternalInput")
    ved = nc.dram_tensor("vep", [128, NTT * HD], bf16, kind="ExternalInput")
    trid = nc.dram_tensor("tri", [128, 384], bf16, kind="ExternalInput")
    outd = nc.dram_tensor("outp", [128, NTT, T], bf16, kind="ExternalOutput")

    with tile.TileContext(nc) as tc, ExitStack() as ctx:
        res = ctx.enter_context(tc.tile_pool(name="res", bufs=1))
        xc_p = ctx.enter_context(tc.tile_pool(name="xc", bufs=2))
        sq_p = ctx.enter_context(tc.tile_pool(name="sq", bufs=2))
        row_p = ctx.enter_context(tc.tile_pool(name="rows", bufs=2))
        bc_p = ctx.enter_context(tc.tile_pool(name="bc", bufs=3))
        qt_p = ctx.enter_context(tc.tile_pool(name="qt", bufs=2))
        es_p = ctx.enter_context(tc.tile_pool(name="es", bufs=4))
        yt_p = ctx.enter_context(tc.tile_pool(name="yt", bufs=2))
        work_p = ctx.enter_context(tc.tile_pool(name="work", bufs=2))
        ot_p = ctx.enter_context(tc.tile_pool(name="ot", bufs=3))

        # PSUM: 8 banks total. qkv(2: k then v, then cproj co rotation)
        # + s(3: q h0/h2, then the attention S pipeline) + out(2: q h1/h3,
        # then attention ps_out rotation) + misc(1: one bank shared serially
        # by the rms row sums (row 0) and the softmax denominators (rows
        # 0/32/64; matmul out bases are limited to 0/32/64)).
        ps_qkv = ctx.enter_context(tc.tile_pool(name="ps_qkv", bufs=2, space="PSUM"))
        ps_s = ctx.enter_context(tc.tile_pool(name="ps_s", bufs=3, space="PSUM"))
        ps_out_p = ctx.enter_context(tc.tile_pool(name="ps_out", bufs=2, space="PSUM"))
        ps_misc = ctx.enter_context(tc.tile_pool(name="ps_misc", bufs=1, space="PSUM"))

        # ---- resident tensors ----
        wq_sb = res.tile([128, NCK, HPC * HD], bf16)
        wk_sb = res.tile([128, NCK, HD], bf16)
        wv_sb = res.tile([128, NCK, HD], bf16)
        wp_sb = res.tile([128, HPC, C], bf16)
        cc_sb = res.tile([128, T], bf16)
        ss_sb = res.tile([128, T], bf16)
        ve_sb = res.tile([128, NTT, HD], bf16)
        tri_sb = res.tile([128, 384], bf16)  # [Mc^T | Me^T | identity]
        kT_sb = res.tile([128, T], bf16)        # rotated+normalized k, HD on partitions
        vn_sb = res.tile([128, NTT, HD], bf16)  # v natural, token tiles on partitions
        ones_sb = res.tile([128, 1], bf16)
        nc.vector.memset(ones_sb, 1.0)
        bq_sb = res.tile([1, 1], f32)
        nc.vector.memset(bq_sb, B_Q)
        bk_sb = res.tile([1, 1], f32)
        nc.vector.memset(bk_sb, B_K)

        xcs = {}
        auxs = {}

        def emit_xload(m4):
            xc = xc_p.tile([128, NCK, TS], bf16, tag="xc")
            t0 = m4 * TS
            for g in range(4):
                nc.sync.dma_start(out=xc[:, 4 * g:4 * g + 4, :],
                                  in_=xd[:, 4 * g:4 * g + 4, t0:t0 + TS])
            xcs[m4] = xc

        def rope(dst, t0):
            """In-place RoPE on a [128, TS] bf16 tile; halves swap via DVE
            reads at a shifted partition base. ss rows 64:128 hold sin, rows
            0:64 hold -sin, matching each op's shared input base."""
            tmp = work_p.tile([128, TS], bf16, tag="tmp")
            nc.vector.tensor_mul(tmp[0:64, :], dst[64:128, :], ss_sb[64:128, t0:t0 + TS])
            nc.vector.tensor_mul(tmp[64:128, :], dst[0:64, :], ss_sb[0:64, t0:t0 + TS])
            nc.vector.tensor_mul(dst, dst, cc_sb[:, t0:t0 + TS])
            nc.vector.tensor_add(dst, dst, tmp)

        def emit_kvproj(m4):
            t0 = m4 * TS
            xc = xcs[m4]
            # k chunk loop, then v in tt-major order: each start=True marks the
            # whole 2048B zero region (= full bank row) pending-zero, so the
            # four v accumulation regions sharing one bank must run strictly
            # one after another — interleaving them clobbers siblings.
            ps_k = ps_qkv.tile([128, TS], f32, tag="qkv")
            for c in range(NCK):
                nc.tensor.matmul(ps_k, wk_sb[:, c, :], xc[:, c, :],
                                 start=(c == 0), stop=(c == NCK - 1))
            ps_v = ps_qkv.tile([128, TPS, HD], f32, tag="qkv")
            for tt in range(TPS):
                for c in range(NCK):
                    nc.tensor.matmul(ps_v[:, tt, :], xc[:, c, tt * 128:(tt + 1) * 128],
                                     wv_sb[:, c, :], start=(c == 0), stop=(c == NCK - 1))
            aux = ps_misc.tile([65, TS], f32, tag="misc")
            auxs[m4] = aux
            sq_k = sq_p.tile([128, TS], bf16, tag="sq")
            nc.scalar.activation(sq_k, ps_k, AF.Square)
            nc.tensor.matmul(aux[0:1, :], ones_sb, sq_k, start=True, stop=True)
            srk = row_p.tile([1, TS], f32, tag="srk")
            nc.scalar.activation(srk, aux[0:1, :], AF.Ln, bias=bk_sb, scale=S_K)
            nc.scalar.activation(srk, srk, AF.Exp, scale=-0.5)
            rbk = bc_p.tile([128, TS], f32, tag="bc")
            nc.gpsimd.partition_broadcast(rbk, srk)
            k_sl = kT_sb[:, t0:t0 + TS]
            nc.vector.tensor_mul(k_sl, ps_k, rbk)
            rope(k_sl, t0)
            for tt in range(TPS):
                nc.vector.tensor_add(vn_sb[:, m4 * TPS + tt, :], ps_v[:, tt, :],
                                     ve_sb[:, m4 * TPS + tt, :])

        def emit_qproj(m4):
            t0 = m4 * TS
            xc = xcs[m4]
            aux = auxs[m4]
            qts = []
            for h in range(HPC):
                pool = ps_s if h % 2 == 0 else ps_out_p
                tag = "s" if h % 2 == 0 else "out"
                ps_q = pool.tile([128, TS], f32, tag=tag)
                for c in range(NCK):
                    nc.tensor.matmul(ps_q, wq_sb[:, c, h * HD:(h + 1) * HD], xc[:, c, :],
                                     start=(c == 0), stop=(c == NCK - 1))
                sq_q = sq_p.tile([128, TS], bf16, tag="sq")
                nc.scalar.activation(sq_q, ps_q, AF.Square)
                # the single aux row is serially reused by k and all q heads:
                # each row is consumed by the Ln activation ~1.2us after it is
                # written, long before the next head's ones-matmul lands.
                nc.tensor.matmul(aux[0:1, :], ones_sb, sq_q, start=True, stop=True)
                srow = row_p.tile([1, TS], f32, tag="srow")
                nc.scalar.activation(srow, aux[0:1, :], AF.Ln,
                                     bias=bq_sb, scale=S_Q)
                nc.scalar.activation(srow, srow, AF.Exp, scale=-0.5)
                rbc = bc_p.tile([128, TS], f32, tag="bc")
                nc.gpsimd.partition_broadcast(rbc, srow)
                qt = qt_p.tile([128, TS], bf16, tag=f"qt{h}")
                nc.vector.tensor_mul(qt, ps_q, rbc)
                rope(qt, t0)
                qts.append(qt)
            return qts

        def make_cproj_fillers(m4, yts, tail=False):
            """One thunk per c_proj matmul of slice m4 (64 total), in co-major
            order; each accumulates into a rotating ps_qkv bank, evacuates
            at h==3, and DMAs out each finished group of 4 co's. For the
            kernel tail the last group streams out per-co so the final DMA
            only carries 128KB."""
            t0 = m4 * TS
            state = {}
            fillers = []

            def make(co, h):
                def f():
                    percol = tail and co >= NTT - 4
                    if h == 0 and co % 4 == 0:
                        state["ot"] = ot_p.tile([128, 4, TS], bf16, tag="ot",
                                                name="ot")
                    if h == 0:
                        state[co] = ps_qkv.tile([128, TS], f32, tag="qkv",
                                                name="ps_p")
                    nc.tensor.matmul(state[co],
                                     wp_sb[:, h, co * 128:(co + 1) * 128],
                                     yts[h], start=(h == 0), stop=(h == HPC - 1))
                    if h == HPC - 1:
                        nc.vector.tensor_copy(state["ot"][:, co % 4, :], state[co])
                        if percol:
                            nc.sync.dma_start(
                                out=outd[:, co:co + 1, t0:t0 + TS],
                                in_=state["ot"][:, co % 4:co % 4 + 1, :])
                        elif co % 4 == 3:
                            cg = co // 4
                            nc.sync.dma_start(
                                out=outd[:, 4 * cg:4 * cg + 4, t0:t0 + TS],
                                in_=state["ot"])
                return f

            if tail:
                # interleave the first two co's h-loops across the two banks
                # so the PE has six matmuls in hand before it needs yt3 (the
                # recip->broadcast->mul chain of the last head).
                for h in range(HPC):
                    fillers.append(make(0, h))
                    fillers.append(make(1, h))
                start_co = 2
            else:
                start_co = 0
            for co in range(start_co, NTT):
                for h in range(HPC):
                    fillers.append(make(co, h))
            return fillers

        def emit_attn(m4, qts, fillers):
            tiles = _ktiles(m4, W)
            last = len(tiles) - 1
            wts = [3 if i == 0 else (2 if i == 1 else 1) for i in range(len(tiles))]
            wsum = sum(wts) * HPC
            nfill = len(fillers)
            fi = 0
            slot = 0
            # denominators live in the same bank as the rms rows: heads 0-2
            # get rows 0/32/64, head 3 reuses row 0 (h0's recip read happened
            # two head-periods earlier).
            sum4 = auxs[m4]
            sum_row = [0, 32, 64, 0]
            yts = []
            for h in range(HPC):
                sr = sum_row[h]
                ps_out = ps_out_p.tile([128, TS], f32, tag="out")
                for idx, (n, f0, f1, cb, eb) in enumerate(tiles):
                    pss = ps_s.tile([128, TS], f32, tag="s")
                    masked = (cb is not None) + (eb is not None)
                    nc.tensor.matmul(pss[:, f0:f1], kT_sb[:, n * 128:(n + 1) * 128],
                                     qts[h][:, f0:f1], start=True, stop=(masked == 0))
                    # boundary masking as a pre-exp -100 additive matmul
                    # (mask^T stationary, identity moving) in the same PSUM
                    # group: exp then yields exact zeros and nothing waits on
                    # the Pool engine.
                    if cb is not None:
                        masked -= 1
                        nc.tensor.matmul(pss[:, cb:cb + 128], tri_sb[:, 0:128],
                                         tri_sb[:, 256:384], start=False,
                                         stop=(masked == 0))
                    if eb is not None:
                        masked -= 1
                        nc.tensor.matmul(pss[:, eb:eb + 128], tri_sb[:, 128:256],
                                         tri_sb[:, 256:384], start=False,
                                         stop=(masked == 0))
                    # place c_proj filler matmuls right after the S matmul:
                    # the PE chews these while the ACT engine runs exp(i).
                    # Pacing is weighted 3/2/1 toward the first tiles of each
                    # head — the exp pipeline needs runway there.
                    slot += wts[idx]
                    while fi < min(nfill, nfill * slot // wsum):
                        fillers[fi]()
                        fi += 1
                    es = es_p.tile([128, TS], bf16, tag="es")
                    nc.scalar.activation(es[:, f0:f1], pss[:, f0:f1], AF.Exp)
                    nc.tensor.matmul(sum4[sr:sr + 1, f0:f1], ones_sb,
                                     es[:, f0:f1], start=(idx == 0), stop=(idx == last))
                    nc.tensor.matmul(ps_out[:, f0:f1], vn_sb[:, n, :], es[:, f0:f1],
                                     start=(idx == 0), stop=(idx == last))
                rsum = row_p.tile([1, TS], f32, tag="rsum")
                nc.vector.reciprocal(rsum, sum4[sr:sr + 1, :])
                sbc = bc_p.tile([128, TS], f32, tag="bc")
                nc.gpsimd.partition_broadcast(sbc, rsum)
                yt = yt_p.tile([128, TS], bf16, tag=f"yt{h}")
                nc.vector.tensor_mul(yt, ps_out, sbc)
                yts.append(yt)
            while fi < nfill:
                fillers[fi]()
                fi += 1
            return yts

        # ---- prologue ----
        # x groups stream on the SP HWDGE queue. wk/wv ride the Activation
        # HWDGE queue so their issue overhead overlaps the x stream (they are
        # needed first and the ACT engine is idle this early); everything
        # else stays on SP behind the x groups, ordered by first use.
        emit_xload(0)
        nc.scalar.dma_start(out=wk_sb, in_=wkd[:, :].rearrange("p (c h) -> p c h", c=NCK))
        nc.scalar.dma_start(out=wv_sb, in_=wvd[:, :].rearrange("p (c h) -> p c h", c=NCK))
        nc.sync.dma_start(out=wq_sb, in_=wqd[:, :].rearrange("p (c h) -> p c h", c=NCK))
        nc.sync.dma_start(out=cc_sb, in_=ccd[:, :])
        nc.sync.dma_start(out=ss_sb, in_=ssd[:, :])
        nc.sync.dma_start(out=ve_sb, in_=ved[:, :].rearrange("p (n h) -> p n h", n=NTT))
        nc.sync.dma_start(out=tri_sb, in_=trid[:, :])
        nc.sync.dma_start(out=wp_sb, in_=wpd[:, :].rearrange("p (g o) -> p g o", g=HPC))

        # ---- software-pipelined slice loop ----
        emit_kvproj(0)
        prev = None
        for m4 in range(NSL):
            if m4 + 1 < NSL:
                emit_xload(m4 + 1)
            qts = emit_qproj(m4)
            fillers = make_cproj_fillers(*prev) if prev else []
            yts = emit_attn(m4, qts, fillers)
            if m4 + 1 < NSL:
                emit_kvproj(m4 + 1)
            prev = (m4, yts)
        for f in make_cproj_fillers(*prev, tail=True):
            f()

    # Restrict the activation-table picker to the one set containing every
    # ACT function we use (exp, ln, square, copy, identity): without this the
    # greedy picker alternates exp_and_others <-> natural_log, inserting a
    # ~1.3us table load per switch. Set ids are positions in act_info.json's
    # list, so unwanted sets are emptied rather than removed.
    import concourse.hw_specs as hw_specs
    import concourse.bacc as bacc_mod

    orig = hw_specs.get_activation_tables

    def only_combined(arch):
        t = orig(arch)
        return {k: (v if k == "natural_log_exp_and_others" else set())
                for k, v in t.items()}

    hw_specs.get_activation_tables = only_combined
    bacc_mod.get_activation_tables = only_combined
    try:
        nc.compile()
    finally:
        hw_specs.get_activation_tables = orig
        bacc_mod.get_activation_tables = orig
    return nc


def _pack_rows(a):
    """[C_rows, F] -> [128, (C_rows/128) * F] SBUF layout (partition-major)."""
    from ml_dtypes import bfloat16

    rows, f = a.shape
    nck = rows // 128
    return np.ascontiguousarray(
        a.reshape(nck, 128, f).transpose(1, 0, 2).reshape(128, nck * f)
    ).astype(bfloat16)


def _prep_inputs(x, ve, cos, sin, Wq, Wk, Wv, Wproj, Wgate, W):
    from ml_dtypes import bfloat16

    cosT = cos[0, :, 0, :].T  # (64, T)
    sinT = sin[0, :, 0, :].T
    cc = np.concatenate([cosT, cosT], axis=0).astype(bfloat16)
    # rows 0:64 = -sin (used by tmp[64:128] = x1 * -sin, both operands base 0),
    # rows 64:128 = sin (used by tmp[0:64] = x2 * sin, both operands base 64).
    ss = np.concatenate([-sinT, sinT], axis=0).astype(bfloat16)
    # Additive -100 masks, pre-transposed for use as the matmul stationary
    # (out[p,f] += Mx^T[f,p] via an identity moving operand):
    #   causal block valid iff p <= f; edge block valid iff f <= p + W%128.
    p = np.arange(128)[:, None]
    f = np.arange(128)[None, :]
    mc = np.where(p <= f, 0.0, -100.0).astype(np.float32).T
    me = np.where(f <= p + (W % 128), 0.0, -100.0).astype(np.float32).T
    ident = np.eye(128, dtype=np.float32)
    tri = np.concatenate([mc, me, ident], axis=1).astype(bfloat16)

    xp_by_b = {}
    for b in range(B):
        xp_by_b[b] = _pack_rows(x[b].T)  # [128, NCK*T]

    in_maps = []
    for core in range(8):
        b, g = core // NKV, core % NKV
        hs = slice(g * HPC * HD, (g + 1) * HPC * HD)
        ks = slice(g * HD, (g + 1) * HD)
        gate = 3.0 / (1.0 + np.exp(-(x[b][:, :GATE_CH] @ Wgate[g])))  # (T,)
        vep = gate[:, None] * ve[b][:, ks]  # (T, HD)
        in_maps.append({
            "xp": xp_by_b[b],
            "wqp": _pack_rows(Wq[hs, :].T),
            "wkp": _pack_rows(Wk[ks, :].T),
            "wvp": _pack_rows(Wv[ks, :].T),
            "wpp": _pack_rows(Wproj[:, hs].T),
            "cc": cc,
            "ss": ss,
            "vep": _pack_rows(vep),
            "tri": tri,
        })
    return in_maps


def _run(inputs, trace=False):
    from concourse.bass_utils import run_bass_kernel_spmd

    x = np.asarray(inputs["x"], dtype=np.float32)
    ve = np.asarray(inputs["ve"], dtype=np.float32)
    cos = np.asarray(inputs["cos"], dtype=np.float32)
    sin = np.asarray(inputs["sin"], dtype=np.float32)
    Wq = np.asarray(inputs["Wq"], dtype=np.float32)
    Wk = np.asarray(inputs["Wk"], dtype=np.float32)
    Wv = np.asarray(inputs["Wv"], dtype=np.float32)
    Wproj = np.asarray(inputs["Wproj"], dtype=np.float32)
    Wgate = np.asarray(inputs["Wgate"], dtype=np.float32)
    W = int(inputs["window_size"])

    if W not in _compiled:
        _compiled[W] = _build(W)
    nc = _compiled[W]

    in_maps = _prep_inputs(x, ve, cos, sin, Wq, Wk, Wv, Wproj, Wgate, W)
    res = run_bass_kernel_spmd(nc, in_maps, core_ids=list(range(8)), trace=trace)

    out = np.zeros((B, T, C), dtype=np.float32)
    for core in range(8):
        b = core // NKV
        # outp: [128, NTT, T] with (p, co, t) = partial[co*128+p, t]
        part = np.asarray(res.results[core]["outp"]).astype(np.float32)
        out[b] += part.transpose(1, 0, 2).reshape(C, T).T
    return out, res


def kernel(**inputs):
    out, _ = _run(inputs, trace=False)
    return out


# revision 29
# speedup vs baseline: 1.0330x; 1.0001x over previous
"""Sliding-window causal self-attention (GQA + RoPE + QK-RMSNorm + ve-gate) on
8 Trainium2 NeuronCores.

Sharding: core c handles (batch b = c // 4, kv-head g = c % 4): data parallel
over batch x tensor parallel over the 4 KV head groups (4 query heads per
core). Each core computes its partial c_proj output; the all-reduce over the 4
head shards is a host-side sum.

v3 design (per core):
  - everything the PE touches is bf16 (inputs are host-converted); PSUM
    accumulation stays fp32, so matmul error is input-quantization only.
  - the ve gate (3*sigmoid(x[:,:12] @ Wgate)) is folded into ve on the host:
    ve' = gate * ve, so the device only does v += ve'.
  - k's rms-norm is folded into kT at PSUM evacuation (broadcast row * PSUM),
    so exp() needs no per-key scale and there are no DRAM round trips.
  - v is computed directly in natural (token-partition) layout by using the
    x chunk as the matmul stationary operand: no PE transposes. The k and v
    chunk loops are fused so slice-0 projections track the x DMA arrivals.
  - RoPE's half-swap uses DVE reads at a shifted partition base (the ss table
    is laid out [-sin; sin] so both SBUF inputs of each TensorTensor share a
    partition base, which the BIR verifier requires).
  - scores are computed transposed (S^T: tk x tq); softmax denominators come
    from a ones-stationary matmul into a shared [97, TS] PSUM tile (rows at
    32h: matmul outputs must start at partition 0/32/64/96); no
    max-subtraction (QK rms-norm bounds |score| <= 1.44*sqrt(128)); masking
    multiplies boundary tiles by 0/1 triangles on the Pool engine.
  - c_proj of slice m-1 is interleaved as single-matmul fillers between the
    attention tiles of slice m: the in-order PE would otherwise park at
    sum(i) waiting for exp(i) on the ACT engine (ACT is 2x slower per column
    than the PE).
  - q-head projections alternate between two PSUM pools so the
    square->rownorm->broadcast->evac chain of head h never blocks head h+1.
  - DMA count is ~41 (vs 251 in the original): weights/tables are
    host-prepacked into SBUF layout ([128, free]) so each is one
    large-descriptor DMA; x streams in 4 group-DMAs per 512-token slice;
    output streams out in 4 group-DMAs per slice (bf16 partials, host sums
    in fp32).
"""

import sys

sys.path.insert(0, "/opt/trn_rl_repo")

import numpy as np

B, T, C = 2, 2048, 2048
NH, NKV, HD = 16, 4, 128
GATE_CH = 12
HPC = NH // NKV          # q heads per core
TS = 512                 # token-slice width
NSL = T // TS            # 4 slices
NCK = C // 128           # 16 contraction chunks
TPS = TS // 128          # 4 token tiles per slice
NTT = T // 128           # 16 token tiles
EPS = 1e-6

A_Q = 1.2 / np.sqrt(float(HD))   # rms-norm scale folded into q (incl 1/sqrt(HD))
A_K = 1.2                        # rms-norm scale folded into k
S_Q = float(1.0 / (HD * A_Q * A_Q))
B_Q = float(EPS / (A_Q * A_Q))
S_K = float(1.0 / (HD * A_K * A_K))
B_K = float(EPS / (A_K * A_K))

_compiled = {}


def _ktiles(m4, W):
    """k-tiles overlapping q-slice m4 with their valid tq-column extents.

    Returns list of (n, f0, f1, causal_block_col, edge_block_col); columns are
    relative to the slice (0..TS). First entry covers [0, TS) fully (it opens
    the PSUM accumulation group).
    """
    assert W % 128 == 0 and W >= 384
    out = []
    for n in range(0, TPS * m4 + TPS):
        f0 = max(0, 128 * n - TS * m4)
        f1 = min(TS, 128 * n + W + 128 - TS * m4)
        if f1 <= f0:
            continue
        causal = 128 * n >= TS * m4            # diagonal staircase inside tile
        edge = (128 * n + W + 128 - TS * m4) <= TS  # window lower edge inside
        cb = f0 if causal else None
        eb = (f1 - 128) if edge else None
        out.append((n, f0, f1, cb, eb))
    full = [e for e in out if e[1] == 0 and e[2] == TS]
    assert full, "need one full-extent tile to open the PSUM group"
    first = full[0]
    rest = [e for e in out if e[0] != first[0]]
    return [first] + rest


def _build(W):
    import concourse.bass as bass
    import concourse.tile as tile
    from concourse import bacc, mybir
    from contextlib import ExitStack

    f32 = mybir.dt.float32
    bf16 = mybir.dt.bfloat16
    AF = mybir.ActivationFunctionType

    nc = bacc.Bacc(None, target_bir_lowering=False)

    xd = nc.dram_tensor("xp", [128, NCK, T], bf16, kind="ExternalInput")
    wqd = nc.dram_tensor("wqp", [128, NCK * HPC * HD], bf16, kind="ExternalInput")
    wkd = nc.dram_tensor("wkp", [128, NCK * HD], bf16, kind="ExternalInput")
    wvd = nc.dram_tensor("wvp", [128, NCK * HD], bf16, kind="ExternalInput")
    wpd = nc.dram_tensor("wpp", [128, HPC * C], bf16, kind="ExternalInput")
    ccd = nc.dram_tensor("cc", [128, T], bf16, kind="ExternalInput")
    ssd = nc.dram_tensor("ss", [128, T], bf16, kind="ExternalInput")
    ved = nc.dram_tensor("vep", [128, NTT * HD], bf16, kind="ExternalInput")
    trid = nc.dram_tensor("tri", [128, 384], bf16, kind="ExternalInput")
    outd = nc.dram_tensor("outp", [128, NTT, T], bf16, kind="ExternalOutput")

    with tile.TileContext(nc) as tc, ExitStack() as ctx:
        res = ctx.enter_context(tc.tile_pool(name="res", bufs=1))
        xc_p = ctx.enter_context(tc.tile_pool(name="xc", bufs=2))
        sq_p = ctx.enter_context(tc.tile_pool(name="sq", bufs=2))
        row_p = ctx.enter_context(tc.tile_pool(name="rows", bufs=2))
        bc_p = ctx.enter_context(tc.tile_pool(name="bc", bufs=3))
        qt_p = ctx.enter_context(tc.tile_pool(name="qt", bufs=2))
        es_p = ctx.enter_context(tc.tile_pool(name="es", bufs=6))
        yt_p = ctx.enter_context(tc.tile_pool(name="yt", bufs=2))
        work_p = ctx.enter_context(tc.tile_pool(name="work", bufs=2))
        ot_p = ctx.enter_context(tc.tile_pool(name="ot", bufs=3))

        # PSUM: 8 banks total. qkv(2: k then v, then cproj co rotation)
        # + s(3: q h0/h2, then the attention S pipeline) + out(2: q h1/h3,
        # then attention ps_out rotation) + misc(1: one bank shared serially
        # by the rms row sums (row 0) and the softmax denominators (rows
        # 0/32/64; matmul out bases are limited to 0/32/64)).
        ps_qkv = ctx.enter_context(tc.tile_pool(name="ps_qkv", bufs=2, space="PSUM"))
        ps_s = ctx.enter_context(tc.tile_pool(name="ps_s", bufs=3, space="PSUM"))
        ps_out_p = ctx.enter_context(tc.tile_pool(name="ps_out", bufs=2, space="PSUM"))
        ps_misc = ctx.enter_context(tc.tile_pool(name="ps_misc", bufs=1, space="PSUM"))

        # ---- resident tensors ----
        wq_sb = res.tile([128, NCK, HPC * HD], bf16)
        wk_sb = res.tile([128, NCK, HD], bf16)
        wv_sb = res.tile([128, NCK, HD], bf16)
        wp_sb = res.tile([128, HPC, C], bf16)
        cc_sb = res.tile([128, T], bf16)
        ss_sb = res.tile([128, T], bf16)
        ve_sb = res.tile([128, NTT, HD], bf16)
        tri_sb = res.tile([128, 384], bf16)  # [Mc^T | Me^T | identity]
        kT_sb = res.tile([128, T], bf16)        # rotated+normalized k, HD on partitions
        vn_sb = res.tile([128, NTT, HD], bf16)  # v natural, token tiles on partitions
        ones_sb = res.tile([128, 1], bf16)
        nc.vector.memset(ones_sb, 1.0)
        bq_sb = res.tile([1, 1], f32)
        nc.vector.memset(bq_sb, B_Q)
        bk_sb = res.tile([1, 1], f32)
        nc.vector.memset(bk_sb, B_K)

        xcs = {}
        auxs = {}

        def emit_xload(m4):
            xc = xc_p.tile([128, NCK, TS], bf16, tag="xc")
            t0 = m4 * TS
            for g in range(4):
                nc.sync.dma_start(out=xc[:, 4 * g:4 * g + 4, :],
                                  in_=xd[:, 4 * g:4 * g + 4, t0:t0 + TS])
            xcs[m4] = xc

        def rope(dst, t0):
            """In-place RoPE on a [128, TS] bf16 tile; halves swap via DVE
            reads at a shifted partition base. ss rows 64:128 hold sin, rows
            0:64 hold -sin, matching each op's shared input base."""
            tmp = work_p.tile([128, TS], bf16, tag="tmp")
            nc.vector.tensor_mul(tmp[0:64, :], dst[64:128, :], ss_sb[64:128, t0:t0 + TS])
            nc.vector.tensor_mul(tmp[64:128, :], dst[0:64, :], ss_sb[0:64, t0:t0 + TS])
            nc.vector.tensor_mul(dst, dst, cc_sb[:, t0:t0 + TS])
            nc.vector.tensor_add(dst, dst, tmp)

        def emit_kvproj(m4):
            t0 = m4 * TS
            xc = xcs[m4]
            # k chunk loop, then v in tt-major order: each start=True marks the
            # whole 2048B zero region (= full bank row) pending-zero, so the
            # four v accumulation regions sharing one bank must run strictly
            # one after another — interleaving them clobbers siblings.
            ps_k = ps_qkv.tile([128, TS], f32, tag="qkv")
            for c in range(NCK):
                nc.tensor.matmul(ps_k, wk_sb[:, c, :], xc[:, c, :],
                                 start=(c == 0), stop=(c == NCK - 1))
            ps_v = ps_qkv.tile([128, TPS, HD], f32, tag="qkv")
            for tt in range(TPS):
                for c in range(NCK):
                    nc.tensor.matmul(ps_v[:, tt, :], xc[:, c, tt * 128:(tt + 1) * 128],
                                     wv_sb[:, c, :], start=(c == 0), stop=(c == NCK - 1))
            aux = ps_misc.tile([65, TS], f32, tag="misc")
            auxs[m4] = aux
            sq_k = sq_p.tile([128, TS], bf16, tag="sq")
            nc.scalar.activation(sq_k, ps_k, AF.Square)
            nc.tensor.matmul(aux[0:1, :], ones_sb, sq_k, start=True, stop=True)
            srk = row_p.tile([1, TS], f32, tag="srk")
            nc.scalar.activation(srk, aux[0:1, :], AF.Ln, bias=bk_sb, scale=S_K)
            nc.scalar.activation(srk, srk, AF.Exp, scale=-0.5)
            rbk = bc_p.tile([128, TS], f32, tag="bc")
            nc.gpsimd.partition_broadcast(rbk, srk)
            k_sl = kT_sb[:, t0:t0 + TS]
            nc.vector.tensor_mul(k_sl, ps_k, rbk)
            rope(k_sl, t0)
            for tt in range(TPS):
                nc.vector.tensor_add(vn_sb[:, m4 * TPS + tt, :], ps_v[:, tt, :],
                                     ve_sb[:, m4 * TPS + tt, :])

        def emit_qproj(m4):
            t0 = m4 * TS
            xc = xcs[m4]
            aux = auxs[m4]
            qts = []
            for h in range(HPC):
                pool = ps_s if h % 2 == 0 else ps_out_p
                tag = "s" if h % 2 == 0 else "out"
                ps_q = pool.tile([128, TS], f32, tag=tag)
                for c in range(NCK):
                    nc.tensor.matmul(ps_q, wq_sb[:, c, h * HD:(h + 1) * HD], xc[:, c, :],
                                     start=(c == 0), stop=(c == NCK - 1))
                sq_q = sq_p.tile([128, TS], bf16, tag="sq")
                nc.scalar.activation(sq_q, ps_q, AF.Square)
                # the single aux row is serially reused by k and all q heads:
                # each row is consumed by the Ln activation ~1.2us after it is
                # written, long before the next head's ones-matmul lands.
                nc.tensor.matmul(aux[0:1, :], ones_sb, sq_q, start=True, stop=True)
                srow = row_p.tile([1, TS], f32, tag="srow")
                nc.scalar.activation(srow, aux[0:1, :], AF.Ln,
                                     bias=bq_sb, scale=S_Q)
                nc.scalar.activation(srow, srow, AF.Exp, scale=-0.5)
                rbc = bc_p.tile([128, TS], f32, tag="bc")
                nc.gpsimd.partition_broadcast(rbc, srow)
                qt = qt_p.tile([128, TS], bf16, tag=f"qt{h}")
                nc.vector.tensor_mul(qt, ps_q, rbc)
                rope(qt, t0)
                qts.append(qt)
            return qts

        def make_cproj_fillers(m4, yts, tail=False):
            """One thunk per c_proj matmul of slice m4 (64 total), in co-major
            order; each accumulates into a rotating ps_qkv bank, evacuates
            at h==3, and DMAs out each finished group of 4 co's. For the
            kernel tail the last group streams out per-co so the final DMA
            only carries 128KB."""
            t0 = m4 * TS
            state = {}
            fillers = []

            def make(co, h):
                def f():
                    percol = tail and co >= NTT - 4
                    if h == 0 and co % 4 == 0:
                        state["ot"] = ot_p.tile([128, 4, TS], bf16, tag="ot",
                                                name="ot")
                    if h == 0:
                        state[co] = ps_qkv.tile([128, TS], f32, tag="qkv",
                                                name="ps_p")
                    nc.tensor.matmul(state[co],
                                     wp_sb[:, h, co * 128:(co + 1) * 128],
                                     yts[h], start=(h == 0), stop=(h == HPC - 1))
                    if h == HPC - 1:
                        nc.vector.tensor_copy(state["ot"][:, co % 4, :], state[co])
                        if percol:
                            nc.sync.dma_start(
                                out=outd[:, co:co + 1, t0:t0 + TS],
                                in_=state["ot"][:, co % 4:co % 4 + 1, :])
                        elif co % 4 == 3:
                            cg = co // 4
                            nc.sync.dma_start(
                                out=outd[:, 4 * cg:4 * cg + 4, t0:t0 + TS],
                                in_=state["ot"])
                return f

            if tail:
                # interleave the first two co's h-loops across the two banks
                # so the PE has six matmuls in hand before it needs yt3 (the
                # recip->broadcast->mul chain of the last head).
                for h in range(HPC):
                    fillers.append(make(0, h))
                    fillers.append(make(1, h))
                start_co = 2
            else:
                start_co = 0
            for co in range(start_co, NTT):
                for h in range(HPC):
                    fillers.append(make(co, h))
            return fillers

        def emit_attn(m4, qts, fillers):
            tiles = _ktiles(m4, W)
            last = len(tiles) - 1
            # filler pacing weights: extra runway at head starts and on masked
            # tiles (whose exp chain includes the extra mask matmul + a small
            # extent, so the PE would otherwise catch up and park).
            wts = [(3 if i == 0 else (2 if i == 1 else 1)) +
                   (1 if (t[3] is not None or t[4] is not None) else 0)
                   for i, t in enumerate(tiles)]
            wsum = sum(wts) * HPC
            nfill = len(fillers)
            fi = 0
            slot = 0
            # denominators live in the same bank as the rms rows: heads 0-2
            # get rows 0/32/64, head 3 reuses row 0 (h0's recip read happened
            # two head-periods earlier).
            sum4 = auxs[m4]
            sum_row = [0, 32, 64, 0]
            yts = []
            for h in range(HPC):
                sr = sum_row[h]
                ps_out = ps_out_p.tile([128, TS], f32, tag="out")
                for idx, (n, f0, f1, cb, eb) in enumerate(tiles):
                    pss = ps_s.tile([128, TS], f32, tag="s")
                    masked = (cb is not None) + (eb is not None)
                    nc.tensor.matmul(pss[:, f0:f1], kT_sb[:, n * 128:(n + 1) * 128],
                                     qts[h][:, f0:f1], start=True, stop=(masked == 0))
                    # boundary masking as a pre-exp -100 additive matmul
                    # (mask^T stationary, identity moving) in the same PSUM
                    # group: exp then yields exact zeros and nothing waits on
                    # the Pool engine.
                    if cb is not None:
                        masked -= 1
                        nc.tensor.matmul(pss[:, cb:cb + 128], tri_sb[:, 0:128],
                                         tri_sb[:, 256:384], start=False,
                                         stop=(masked == 0))
                    if eb is not None:
                        masked -= 1
                        nc.tensor.matmul(pss[:, eb:eb + 128], tri_sb[:, 128:256],
                                         tri_sb[:, 256:384], start=False,
                                         stop=(masked == 0))
                    # place c_proj filler matmuls right after the S matmul:
                    # the PE chews these while the ACT engine runs exp(i).
                    # Pacing is weighted 3/2/1 toward the first tiles of each
                    # head — the exp pipeline needs runway there.
                    slot += wts[idx]
                    while fi < min(nfill, nfill * slot // wsum):
                        fillers[fi]()
                        fi += 1
                    es = es_p.tile([128, TS], bf16, tag="es")
                    nc.scalar.activation(es[:, f0:f1], pss[:, f0:f1], AF.Exp)
                    nc.tensor.matmul(sum4[sr:sr + 1, f0:f1], ones_sb,
                                     es[:, f0:f1], start=(idx == 0), stop=(idx == last))
                    nc.tensor.matmul(ps_out[:, f0:f1], vn_sb[:, n, :], es[:, f0:f1],
                                     start=(idx == 0), stop=(idx == last))
                rsum = row_p.tile([1, TS], f32, tag="rsum")
                nc.vector.reciprocal(rsum, sum4[sr:sr + 1, :])
                sbc = bc_p.tile([128, TS], f32, tag="bc")
                nc.gpsimd.partition_broadcast(sbc, rsum)
                yt = yt_p.tile([128, TS], bf16, tag=f"yt{h}")
                nc.vector.tensor_mul(yt, ps_out, sbc)
                yts.append(yt)
            while fi < nfill:
                fillers[fi]()
                fi += 1
            return yts

        # ---- prologue ----
        # x groups stream on the SP HWDGE queue. wk/wv ride the Activation
        # HWDGE queue so their issue overhead overlaps the x stream (they are
        # needed first and the ACT engine is idle this early); everything
        # else stays on SP behind the x groups, ordered by first use.
        emit_xload(0)
        nc.scalar.dma_start(out=wk_sb, in_=wkd[:, :].rearrange("p (c h) -> p c h", c=NCK))
        nc.scalar.dma_start(out=wv_sb, in_=wvd[:, :].rearrange("p (c h) -> p c h", c=NCK))
        nc.sync.dma_start(out=wq_sb, in_=wqd[:, :].rearrange("p (c h) -> p c h", c=NCK))
        nc.sync.dma_start(out=cc_sb, in_=ccd[:, :])
        nc.sync.dma_start(out=ss_sb, in_=ssd[:, :])
        nc.sync.dma_start(out=ve_sb, in_=ved[:, :].rearrange("p (n h) -> p n h", n=NTT))
        nc.sync.dma_start(out=tri_sb, in_=trid[:, :])
        nc.sync.dma_start(out=wp_sb, in_=wpd[:, :].rearrange("p (g o) -> p g o", g=HPC))

        # ---- software-pipelined slice loop ----
        emit_kvproj(0)
        prev = None
        for m4 in range(NSL):
            if m4 + 1 < NSL:
                emit_xload(m4 + 1)
            qts = emit_qproj(m4)
            fillers = make_cproj_fillers(*prev) if prev else []
            yts = emit_attn(m4, qts, fillers)
            if m4 + 1 < NSL:
                emit_kvproj(m4 + 1)
            prev = (m4, yts)
        for f in make_cproj_fillers(*prev, tail=True):
            f()

    # Restrict the activation-table picker to the one set containing every
    # ACT function we use (exp, ln, square, copy, identity): without this the
    # greedy picker alternates exp_and_others <-> natural_log, inserting a
    # ~1.3us table load per switch. Set ids are positions in act_info.json's
    # list, so unwanted sets are emptied rather than removed.
    import concourse.hw_specs as hw_specs
    import concourse.bacc as bacc_mod

    orig = hw_specs.get_activation_tables

    def only_combined(arch):
        t = orig(arch)
        return {k: (v if k == "natural_log_exp_and_others" else set())
                for k, v in t.items()}

    hw_specs.get_activation_tables = only_combined
    bacc_mod.get_activation_tables = only_combined
    try:
        nc.compile()
    finally:
        hw_specs.get_activation_tables = orig
        bacc_mod.get_activation_tables = orig
    return nc


def _pack_rows(a):
    """[C_rows, F] -> [128, (C_rows/128) * F] SBUF layout (partition-major)."""
    from ml_dtypes import bfloat16

    rows, f = a.shape
    nck = rows // 128
    return np.ascontiguousarray(
        a.reshape(nck, 128, f).transpose(1, 0, 2).reshape(128, nck * f)
    ).astype(bfloat16)


def _prep_inputs(x, ve, cos, sin, Wq, Wk, Wv, Wproj, Wgate, W):
    from ml_dtypes import bfloat16

    cosT = cos[0, :, 0, :].T  # (64, T)
    sinT = sin[0, :, 0, :].T
    cc = np.concatenate([cosT, cosT], axis=0).astype(bfloat16)
    # rows 0:64 = -sin (used by tmp[64:128] = x1 * -sin, both operands base 0),
    # rows 64:128 = sin (used by tmp[0:64] = x2 * sin, both operands base 64).
    ss = np.concatenate([-sinT, sinT], axis=0).astype(bfloat16)
    # Additive -100 masks, pre-transposed for use as the matmul stationary
    # (out[p,f] += Mx^T[f,p] via an identity moving operand):
    #   causal block valid iff p <= f; edge block valid iff f <= p + W%128.
    p = np.arange(128)[:, None]
    f = np.arange(128)[None, :]
    mc = np.where(p <= f, 0.0, -100.0).astype(np.float32).T
    me = np.where(f <= p + (W % 128), 0.0, -100.0).astype(np.float32).T
    ident = np.eye(128, dtype=np.float32)
    tri = np.concatenate([mc, me, ident], axis=1).astype(bfloat16)

    xp_by_b = {}
    for b in range(B):
        xp_by_b[b] = _pack_rows(x[b].T)  # [128, NCK*T]

    in_maps = []
    for core in range(8):
        b, g = core // NKV, core % NKV
        hs = slice(g * HPC * HD, (g + 1) * HPC * HD)
        ks = slice(g * HD, (g + 1) * HD)
        gate = 3.0 / (1.0 + np.exp(-(x[b][:, :GATE_CH] @ Wgate[g])))  # (T,)
        vep = gate[:, None] * ve[b][:, ks]  # (T, HD)
        in_maps.append({
            "xp": xp_by_b[b],
            "wqp": _pack_rows(Wq[hs, :].T),
            "wkp": _pack_rows(Wk[ks, :].T),
            "wvp": _pack_rows(Wv[ks, :].T),
            "wpp": _pack_rows(Wproj[:, hs].T),
            "cc": cc,
            "ss": ss,
            "vep": _pack_rows(vep),
            "tri": tri,
        })
    return in_maps


def _run(inputs, trace=False):
    from concourse.bass_utils import run_bass_kernel_spmd

    x = np.asarray(inputs["x"], dtype=np.float32)
    ve = np.asarray(inputs["ve"], dtype=np.float32)
    cos = np.asarray(inputs["cos"], dtype=np.float32)
    sin = np.asarray(inputs["sin"], dtype=np.float32)
    Wq = np.asarray(inputs["Wq"], dtype=np.float32)
    Wk = np.asarray(inputs["Wk"], dtype=np.float32)
    Wv = np.asarray(inputs["Wv"], dtype=np.float32)
    Wproj = np.asarray(inputs["Wproj"], dtype=np.float32)
    Wgate = np.asarray(inputs["Wgate"], dtype=np.float32)
    W = int(inputs["window_size"])

    if W not in _compiled:
        _compiled[W] = _build(W)
    nc = _compiled[W]

    in_maps = _prep_inputs(x, ve, cos, sin, Wq, Wk, Wv, Wproj, Wgate, W)
    res = run_bass_kernel_spmd(nc, in_maps, core_ids=list(range(8)), trace=trace)

    out = np.zeros((B, T, C), dtype=np.float32)
    for core in range(8):
        b = core // NKV
        # outp: [128, NTT, T] with (p, co, t) = partial[co*128+p, t]
        part = np.asarray(res.results[core]["outp"]).astype(np.float32)
        out[b] += part.transpose(1, 0, 2).reshape(C, T).T
    return out, res


def kernel(**inputs):
    out, _ = _run(inputs, trace=False)
    return out


# revision 31
# speedup vs baseline: 1.0419x; 1.0087x over previous
"""Sliding-window causal self-attention (GQA + RoPE + QK-RMSNorm + ve-gate) on
8 Trainium2 NeuronCores.

Sharding: core c handles (batch b = c // 4, kv-head g = c % 4): data parallel
over batch x tensor parallel over the 4 KV head groups (4 query heads per
core). Each core computes its partial c_proj output; the all-reduce over the 4
head shards is a host-side sum.

v3 design (per core):
  - everything the PE touches is bf16 (inputs are host-converted); PSUM
    accumulation stays fp32, so matmul error is input-quantization only.
  - the ve gate (3*sigmoid(x[:,:12] @ Wgate)) is folded into ve on the host:
    ve' = gate * ve, so the device only does v += ve'.
  - k's rms-norm is folded into kT at PSUM evacuation (broadcast row * PSUM),
    so exp() needs no per-key scale and there are no DRAM round trips.
  - v is computed directly in natural (token-partition) layout by using the
    x chunk as the matmul stationary operand: no PE transposes. The k and v
    chunk loops are fused so slice-0 projections track the x DMA arrivals.
  - RoPE's half-swap uses DVE reads at a shifted partition base (the ss table
    is laid out [-sin; sin] so both SBUF inputs of each TensorTensor share a
    partition base, which the BIR verifier requires).
  - scores are computed transposed (S^T: tk x tq); softmax denominators come
    from a ones-stationary matmul into a shared [97, TS] PSUM tile (rows at
    32h: matmul outputs must start at partition 0/32/64/96); no
    max-subtraction (QK rms-norm bounds |score| <= 1.44*sqrt(128)); masking
    multiplies boundary tiles by 0/1 triangles on the Pool engine.
  - c_proj of slice m-1 is interleaved as single-matmul fillers between the
    attention tiles of slice m: the in-order PE would otherwise park at
    sum(i) waiting for exp(i) on the ACT engine (ACT is 2x slower per column
    than the PE).
  - q-head projections alternate between two PSUM pools so the
    square->rownorm->broadcast->evac chain of head h never blocks head h+1.
  - DMA count is ~41 (vs 251 in the original): weights/tables are
    host-prepacked into SBUF layout ([128, free]) so each is one
    large-descriptor DMA; x streams in 4 group-DMAs per 512-token slice;
    output streams out in 4 group-DMAs per slice (bf16 partials, host sums
    in fp32).
"""

import sys

sys.path.insert(0, "/opt/trn_rl_repo")

import numpy as np

B, T, C = 2, 2048, 2048
NH, NKV, HD = 16, 4, 128
GATE_CH = 12
HPC = NH // NKV          # q heads per core
TS = 512                 # token-slice width
NSL = T // TS            # 4 slices
NCK = C // 128           # 16 contraction chunks
TPS = TS // 128          # 4 token tiles per slice
NTT = T // 128           # 16 token tiles
EPS = 1e-6

A_Q = 1.2 / np.sqrt(float(HD))   # rms-norm scale folded into q (incl 1/sqrt(HD))
A_K = 1.2                        # rms-norm scale folded into k
S_Q = float(1.0 / (HD * A_Q * A_Q))
B_Q = float(EPS / (A_Q * A_Q))
S_K = float(1.0 / (HD * A_K * A_K))
B_K = float(EPS / (A_K * A_K))

_compiled = {}


def _ktiles(m4, W):
    """k-tiles overlapping q-slice m4 with their valid tq-column extents.

    Returns list of (n, f0, f1, causal_block_col, edge_block_col); columns are
    relative to the slice (0..TS). First entry covers [0, TS) fully (it opens
    the PSUM accumulation group).
    """
    assert W % 128 == 0 and W >= 384
    out = []
    for n in range(0, TPS * m4 + TPS):
        f0 = max(0, 128 * n - TS * m4)
        f1 = min(TS, 128 * n + W + 128 - TS * m4)
        if f1 <= f0:
            continue
        causal = 128 * n >= TS * m4            # diagonal staircase inside tile
        edge = (128 * n + W + 128 - TS * m4) <= TS  # window lower edge inside
        cb = f0 if causal else None
        eb = (f1 - 128) if edge else None
        out.append((n, f0, f1, cb, eb))
    full = [e for e in out if e[1] == 0 and e[2] == TS]
    assert full, "need one full-extent tile to open the PSUM group"
    first = full[0]
    rest = [e for e in out if e[0] != first[0]]
    return [first] + rest


def _build(W):
    import concourse.bass as bass
    import concourse.tile as tile
    from concourse import bacc, mybir
    from contextlib import ExitStack

    f32 = mybir.dt.float32
    bf16 = mybir.dt.bfloat16
    AF = mybir.ActivationFunctionType

    nc = bacc.Bacc(None, target_bir_lowering=False)

    xd = nc.dram_tensor("xp", [128, NCK, T], bf16, kind="ExternalInput")
    wqd = nc.dram_tensor("wqp", [128, NCK * HPC * HD], bf16, kind="ExternalInput")
    wkd = nc.dram_tensor("wkp", [128, NCK * HD], bf16, kind="ExternalInput")
    wvd = nc.dram_tensor("wvp", [128, NCK * HD], bf16, kind="ExternalInput")
    wpd = nc.dram_tensor("wpp", [128, HPC * C], bf16, kind="ExternalInput")
    ccd = nc.dram_tensor("cc", [128, T], bf16, kind="ExternalInput")
    ssd = nc.dram_tensor("ss", [128, T], bf16, kind="ExternalInput")
    ved = nc.dram_tensor("vep", [128, NTT * HD], bf16, kind="ExternalInput")
    trid = nc.dram_tensor("tri", [128, 384], bf16, kind="ExternalInput")
    outd = nc.dram_tensor("outp", [128, NTT, T], bf16, kind="ExternalOutput")

    with tile.TileContext(nc) as tc, ExitStack() as ctx:
        res = ctx.enter_context(tc.tile_pool(name="res", bufs=1))
        xc_p = ctx.enter_context(tc.tile_pool(name="xc", bufs=2))
        sq_p = ctx.enter_context(tc.tile_pool(name="sq", bufs=2))
        row_p = ctx.enter_context(tc.tile_pool(name="rows", bufs=2))
        bc_p = ctx.enter_context(tc.tile_pool(name="bc", bufs=3))
        qt_p = ctx.enter_context(tc.tile_pool(name="qt", bufs=2))
        es_p = ctx.enter_context(tc.tile_pool(name="es", bufs=6))
        yt_p = ctx.enter_context(tc.tile_pool(name="yt", bufs=2))
        work_p = ctx.enter_context(tc.tile_pool(name="work", bufs=2))
        ot_p = ctx.enter_context(tc.tile_pool(name="ot", bufs=3))

        # PSUM: 8 banks total. qkv(2: k then v, then cproj co rotation)
        # + s(3: q h0/h2, then the attention S pipeline) + out(2: q h1/h3,
        # then attention ps_out rotation) + misc(1: one bank shared serially
        # by the rms row sums (row 0) and the softmax denominators (rows
        # 0/32/64; matmul out bases are limited to 0/32/64)).
        ps_qkv = ctx.enter_context(tc.tile_pool(name="ps_qkv", bufs=2, space="PSUM"))
        ps_s = ctx.enter_context(tc.tile_pool(name="ps_s", bufs=3, space="PSUM"))
        ps_out_p = ctx.enter_context(tc.tile_pool(name="ps_out", bufs=2, space="PSUM"))
        ps_misc = ctx.enter_context(tc.tile_pool(name="ps_misc", bufs=1, space="PSUM"))

        # ---- resident tensors ----
        wq_sb = res.tile([128, NCK, HPC * HD], bf16)
        wk_sb = res.tile([128, NCK, HD], bf16)
        wv_sb = res.tile([128, NCK, HD], bf16)
        wp_sb = res.tile([128, HPC, C], bf16)
        cc_sb = res.tile([128, T], bf16)
        ss_sb = res.tile([128, T], bf16)
        ve_sb = res.tile([128, NTT, HD], bf16)
        tri_sb = res.tile([128, 384], bf16)  # [Mc^T | Me^T | identity]
        kT_sb = res.tile([128, T], bf16)        # rotated+normalized k, HD on partitions
        vn_sb = res.tile([128, NTT, HD], bf16)  # v natural, token tiles on partitions
        ones_sb = res.tile([128, 1], bf16)
        nc.vector.memset(ones_sb, 1.0)
        bq_sb = res.tile([1, 1], f32)
        nc.vector.memset(bq_sb, B_Q)
        bk_sb = res.tile([1, 1], f32)
        nc.vector.memset(bk_sb, B_K)

        xcs = {}
        auxs = {}

        def emit_xload(m4):
            xc = xc_p.tile([128, NCK, TS], bf16, tag="xc")
            t0 = m4 * TS
            for g in range(4):
                nc.sync.dma_start(out=xc[:, 4 * g:4 * g + 4, :],
                                  in_=xd[:, 4 * g:4 * g + 4, t0:t0 + TS])
            xcs[m4] = xc

        def rope(dst, t0):
            """In-place RoPE on a [128, TS] bf16 tile; halves swap via DVE
            reads at a shifted partition base. ss rows 64:128 hold sin, rows
            0:64 hold -sin, matching each op's shared input base."""
            tmp = work_p.tile([128, TS], bf16, tag="tmp")
            nc.vector.tensor_mul(tmp[0:64, :], dst[64:128, :], ss_sb[64:128, t0:t0 + TS])
            nc.vector.tensor_mul(tmp[64:128, :], dst[0:64, :], ss_sb[0:64, t0:t0 + TS])
            nc.vector.tensor_mul(dst, dst, cc_sb[:, t0:t0 + TS])
            nc.vector.tensor_add(dst, dst, tmp)

        def emit_kvproj(m4):
            t0 = m4 * TS
            xc = xcs[m4]
            # k chunk loop, then v in tt-major order: each start=True marks the
            # whole 2048B zero region (= full bank row) pending-zero, so the
            # four v accumulation regions sharing one bank must run strictly
            # one after another — interleaving them clobbers siblings.
            ps_k = ps_qkv.tile([128, TS], f32, tag="qkv")
            for c in range(NCK):
                nc.tensor.matmul(ps_k, wk_sb[:, c, :], xc[:, c, :],
                                 start=(c == 0), stop=(c == NCK - 1))
            ps_v = ps_qkv.tile([128, TPS, HD], f32, tag="qkv")
            for tt in range(TPS):
                for c in range(NCK):
                    nc.tensor.matmul(ps_v[:, tt, :], xc[:, c, tt * 128:(tt + 1) * 128],
                                     wv_sb[:, c, :], start=(c == 0), stop=(c == NCK - 1))
            aux = ps_misc.tile([65, TS], f32, tag="misc")
            auxs[m4] = aux
            sq_k = sq_p.tile([128, TS], bf16, tag="sq")
            nc.scalar.activation(sq_k, ps_k, AF.Square)
            nc.tensor.matmul(aux[0:1, :], ones_sb, sq_k, start=True, stop=True)
            srk = row_p.tile([1, TS], f32, tag="srk")
            nc.scalar.activation(srk, aux[0:1, :], AF.Ln, bias=bk_sb, scale=S_K)
            nc.scalar.activation(srk, srk, AF.Exp, scale=-0.5)
            rbk = bc_p.tile([128, TS], f32, tag="bc")
            nc.gpsimd.partition_broadcast(rbk, srk)
            k_sl = kT_sb[:, t0:t0 + TS]
            nc.vector.tensor_mul(k_sl, ps_k, rbk)
            rope(k_sl, t0)
            for tt in range(TPS):
                nc.vector.tensor_add(vn_sb[:, m4 * TPS + tt, :], ps_v[:, tt, :],
                                     ve_sb[:, m4 * TPS + tt, :])

        def emit_qproj(m4):
            t0 = m4 * TS
            xc = xcs[m4]
            aux = auxs[m4]
            qts = []
            for h in range(HPC):
                pool = ps_s if h % 2 == 0 else ps_out_p
                tag = "s" if h % 2 == 0 else "out"
                ps_q = pool.tile([128, TS], f32, tag=tag)
                for c in range(NCK):
                    nc.tensor.matmul(ps_q, wq_sb[:, c, h * HD:(h + 1) * HD], xc[:, c, :],
                                     start=(c == 0), stop=(c == NCK - 1))
                sq_q = sq_p.tile([128, TS], bf16, tag="sq")
                nc.scalar.activation(sq_q, ps_q, AF.Square)
                # the single aux row is serially reused by k and all q heads:
                # each row is consumed by the Ln activation ~1.2us after it is
                # written, long before the next head's ones-matmul lands.
                nc.tensor.matmul(aux[0:1, :], ones_sb, sq_q, start=True, stop=True)
                srow = row_p.tile([1, TS], f32, tag="srow")
                nc.scalar.activation(srow, aux[0:1, :], AF.Ln,
                                     bias=bq_sb, scale=S_Q)
                nc.scalar.activation(srow, srow, AF.Exp, scale=-0.5)
                rbc = bc_p.tile([128, TS], f32, tag="bc")
                nc.gpsimd.partition_broadcast(rbc, srow)
                qt = qt_p.tile([128, TS], bf16, tag=f"qt{h}")
                nc.vector.tensor_mul(qt, ps_q, rbc)
                rope(qt, t0)
                qts.append(qt)
            return qts

        def make_cproj_fillers(m4, yts, tail=False):
            """One thunk per c_proj matmul of slice m4 (64 total), in co-major
            order; each accumulates into a rotating ps_qkv bank, evacuates
            at h==3, and DMAs out each finished group of 4 co's. For the
            kernel tail the last group streams out per-co so the final DMA
            only carries 128KB."""
            t0 = m4 * TS
            state = {}
            fillers = []

            def make(co, h):
                def f():
                    percol = tail and co >= NTT - 4
                    if h == 0 and co % 4 == 0:
                        state["ot"] = ot_p.tile([128, 4, TS], bf16, tag="ot",
                                                name="ot")
                    if h == 0:
                        # in the kernel tail the attention ps_s banks are
                        # free: borrow one for every third co so the PE has 9
                        # matmuls of runway before the first yt3 wait.
                        if tail and co % 3 == 2:
                            state[co] = ps_s.tile([128, TS], f32, tag="s",
                                                  name="ps_p")
                        else:
                            state[co] = ps_qkv.tile([128, TS], f32, tag="qkv",
                                                    name="ps_p")
                    nc.tensor.matmul(state[co],
                                     wp_sb[:, h, co * 128:(co + 1) * 128],
                                     yts[h], start=(h == 0), stop=(h == HPC - 1))
                    if h == HPC - 1:
                        nc.vector.tensor_copy(state["ot"][:, co % 4, :], state[co])
                        if percol:
                            nc.sync.dma_start(
                                out=outd[:, co:co + 1, t0:t0 + TS],
                                in_=state["ot"][:, co % 4:co % 4 + 1, :])
                        elif co % 4 == 3:
                            cg = co // 4
                            nc.sync.dma_start(
                                out=outd[:, 4 * cg:4 * cg + 4, t0:t0 + TS],
                                in_=state["ot"])
                return f

            if tail:
                # interleave the first three co's h-loops across three banks
                # so the PE has nine matmuls in hand before it needs yt3 (the
                # recip->broadcast->mul chain of the last head).
                for h in range(HPC):
                    for co in range(3):
                        fillers.append(make(co, h))
                start_co = 3
            else:
                start_co = 0
            for co in range(start_co, NTT):
                for h in range(HPC):
                    fillers.append(make(co, h))
            return fillers

        def emit_attn(m4, qts, fillers):
            tiles = _ktiles(m4, W)
            last = len(tiles) - 1
            # filler pacing weights: extra runway at head starts and on masked
            # tiles (whose exp chain includes the extra mask matmul + a small
            # extent, so the PE would otherwise catch up and park).
            wts = [(3 if i == 0 else (2 if i == 1 else 1)) +
                   (1 if (t[3] is not None or t[4] is not None) else 0)
                   for i, t in enumerate(tiles)]
            wsum = sum(wts) * HPC
            nfill = len(fillers)
            fi = 0
            slot = 0
            # denominators live in the same bank as the rms rows: heads 0-2
            # get rows 0/32/64, head 3 reuses row 0 (h0's recip read happened
            # two head-periods earlier).
            sum4 = auxs[m4]
            sum_row = [0, 32, 64, 0]
            yts = []
            for h in range(HPC):
                sr = sum_row[h]
                ps_out = ps_out_p.tile([128, TS], f32, tag="out")
                for idx, (n, f0, f1, cb, eb) in enumerate(tiles):
                    pss = ps_s.tile([128, TS], f32, tag="s")
                    masked = (cb is not None) + (eb is not None)
                    nc.tensor.matmul(pss[:, f0:f1], kT_sb[:, n * 128:(n + 1) * 128],
                                     qts[h][:, f0:f1], start=True, stop=(masked == 0))
                    # boundary masking as a pre-exp -100 additive matmul
                    # (mask^T stationary, identity moving) in the same PSUM
                    # group: exp then yields exact zeros and nothing waits on
                    # the Pool engine.
                    if cb is not None:
                        masked -= 1
                        nc.tensor.matmul(pss[:, cb:cb + 128], tri_sb[:, 0:128],
                                         tri_sb[:, 256:384], start=False,
                                         stop=(masked == 0))
                    if eb is not None:
                        masked -= 1
                        nc.tensor.matmul(pss[:, eb:eb + 128], tri_sb[:, 128:256],
                                         tri_sb[:, 256:384], start=False,
                                         stop=(masked == 0))
                    # place c_proj filler matmuls right after the S matmul:
                    # the PE chews these while the ACT engine runs exp(i).
                    # Pacing is weighted 3/2/1 toward the first tiles of each
                    # head — the exp pipeline needs runway there.
                    slot += wts[idx]
                    while fi < min(nfill, nfill * slot // wsum):
                        fillers[fi]()
                        fi += 1
                    es = es_p.tile([128, TS], bf16, tag="es")
                    nc.scalar.activation(es[:, f0:f1], pss[:, f0:f1], AF.Exp)
                    nc.tensor.matmul(sum4[sr:sr + 1, f0:f1], ones_sb,
                                     es[:, f0:f1], start=(idx == 0), stop=(idx == last))
                    nc.tensor.matmul(ps_out[:, f0:f1], vn_sb[:, n, :], es[:, f0:f1],
                                     start=(idx == 0), stop=(idx == last))
                rsum = row_p.tile([1, TS], f32, tag="rsum")
                nc.vector.reciprocal(rsum, sum4[sr:sr + 1, :])
                sbc = bc_p.tile([128, TS], f32, tag="bc")
                nc.gpsimd.partition_broadcast(sbc, rsum)
                yt = yt_p.tile([128, TS], bf16, tag=f"yt{h}")
                nc.vector.tensor_mul(yt, ps_out, sbc)
                yts.append(yt)
            while fi < nfill:
                fillers[fi]()
                fi += 1
            return yts

        # ---- prologue ----
        # x groups stream on the SP HWDGE queue. wk/wv ride the Activation
        # HWDGE queue so their issue overhead overlaps the x stream (they are
        # needed first and the ACT engine is idle this early); everything
        # else stays on SP behind the x groups, ordered by first use.
        emit_xload(0)
        nc.scalar.dma_start(out=wk_sb, in_=wkd[:, :].rearrange("p (c h) -> p c h", c=NCK))
        nc.scalar.dma_start(out=wv_sb, in_=wvd[:, :].rearrange("p (c h) -> p c h", c=NCK))
        nc.sync.dma_start(out=wq_sb, in_=wqd[:, :].rearrange("p (c h) -> p c h", c=NCK))
        nc.sync.dma_start(out=cc_sb, in_=ccd[:, :])
        nc.sync.dma_start(out=ss_sb, in_=ssd[:, :])
        nc.sync.dma_start(out=ve_sb, in_=ved[:, :].rearrange("p (n h) -> p n h", n=NTT))
        nc.sync.dma_start(out=tri_sb, in_=trid[:, :])
        nc.sync.dma_start(out=wp_sb, in_=wpd[:, :].rearrange("p (g o) -> p g o", g=HPC))

        # ---- software-pipelined slice loop ----
        emit_kvproj(0)
        prev = None
        for m4 in range(NSL):
            if m4 + 1 < NSL:
                emit_xload(m4 + 1)
            qts = emit_qproj(m4)
            fillers = make_cproj_fillers(*prev) if prev else []
            yts = emit_attn(m4, qts, fillers)
            if m4 + 1 < NSL:
                emit_kvproj(m4 + 1)
            prev = (m4, yts)
        for f in make_cproj_fillers(*prev, tail=True):
            f()

    # Restrict the activation-table picker to the one set containing every
    # ACT function we use (exp, ln, square, copy, identity): without this the
    # greedy picker alternates exp_and_others <-> natural_log, inserting a
    # ~1.3us table load per switch. Set ids are positions in act_info.json's
    # list, so unwanted sets are emptied rather than removed.
    import concourse.hw_specs as hw_specs
    import concourse.bacc as bacc_mod

    orig = hw_specs.get_activation_tables

    def only_combined(arch):
        t = orig(arch)
        return {k: (v if k == "natural_log_exp_and_others" else set())
                for k, v in t.items()}

    hw_specs.get_activation_tables = only_combined
    bacc_mod.get_activation_tables = only_combined
    try:
        nc.compile()
    finally:
        hw_specs.get_activation_tables = orig
        bacc_mod.get_activation_tables = orig
    return nc


def _pack_rows(a):
    """[C_rows, F] -> [128, (C_rows/128) * F] SBUF layout (partition-major)."""
    from ml_dtypes import bfloat16

    rows, f = a.shape
    nck = rows // 128
    return np.ascontiguousarray(
        a.reshape(nck, 128, f).transpose(1, 0, 2).reshape(128, nck * f)
    ).astype(bfloat16)


def _prep_inputs(x, ve, cos, sin, Wq, Wk, Wv, Wproj, Wgate, W):
    from ml_dtypes import bfloat16

    cosT = cos[0, :, 0, :].T  # (64, T)
    sinT = sin[0, :, 0, :].T
    cc = np.concatenate([cosT, cosT], axis=0).astype(bfloat16)
    # rows 0:64 = -sin (used by tmp[64:128] = x1 * -sin, both operands base 0),
    # rows 64:128 = sin (used by tmp[0:64] = x2 * sin, both operands base 64).
    ss = np.concatenate([-sinT, sinT], axis=0).astype(bfloat16)
    # Additive -100 masks, pre-transposed for use as the matmul stationary
    # (out[p,f] += Mx^T[f,p] via an identity moving operand):
    #   causal block valid iff p <= f; edge block valid iff f <= p + W%128.
    p = np.arange(128)[:, None]
    f = np.arange(128)[None, :]
    mc = np.where(p <= f, 0.0, -100.0).astype(np.float32).T
    me = np.where(f <= p + (W % 128), 0.0, -100.0).astype(np.float32).T
    ident = np.eye(128, dtype=np.float32)
    tri = np.concatenate([mc, me, ident], axis=1).astype(bfloat16)

    xp_by_b = {}
    for b in range(B):
        xp_by_b[b] = _pack_rows(x[b].T)  # [128, NCK*T]

    in_maps = []
    for core in range(8):
        b, g = core // NKV, core % NKV
        hs = slice(g * HPC * HD, (g + 1) * HPC * HD)
        ks = slice(g * HD, (g + 1) * HD)
        gate = 3.0 / (1.0 + np.exp(-(x[b][:, :GATE_CH] @ Wgate[g])))  # (T,)
        vep = gate[:, None] * ve[b][:, ks]  # (T, HD)
        in_maps.append({
            "xp": xp_by_b[b],
            "wqp": _pack_rows(Wq[hs, :].T),
            "wkp": _pack_rows(Wk[ks, :].T),
            "wvp": _pack_rows(Wv[ks, :].T),
            "wpp": _pack_rows(Wproj[:, hs].T),
            "cc": cc,
            "ss": ss,
            "vep": _pack_rows(vep),
            "tri": tri,
        })
    return in_maps


def _run(inputs, trace=False):
    from concourse.bass_utils import run_bass_kernel_spmd

    x = np.asarray(inputs["x"], dtype=np.float32)
    ve = np.asarray(inputs["ve"], dtype=np.float32)
    cos = np.asarray(inputs["cos"], dtype=np.float32)
    sin = np.asarray(inputs["sin"], dtype=np.float32)
    Wq = np.asarray(inputs["Wq"], dtype=np.float32)
    Wk = np.asarray(inputs["Wk"], dtype=np.float32)
    Wv = np.asarray(inputs["Wv"], dtype=np.float32)
    Wproj = np.asarray(inputs["Wproj"], dtype=np.float32)
    Wgate = np.asarray(inputs["Wgate"], dtype=np.float32)
    W = int(inputs["window_size"])

    if W not in _compiled:
        _compiled[W] = _build(W)
    nc = _compiled[W]

    in_maps = _prep_inputs(x, ve, cos, sin, Wq, Wk, Wv, Wproj, Wgate, W)
    res = run_bass_kernel_spmd(nc, in_maps, core_ids=list(range(8)), trace=trace)

    out = np.zeros((B, T, C), dtype=np.float32)
    for core in range(8):
        b = core // NKV
        # outp: [128, NTT, T] with (p, co, t) = partial[co*128+p, t]
        part = np.asarray(res.results[core]["outp"]).astype(np.float32)
        out[b] += part.transpose(1, 0, 2).reshape(C, T).T
    return out, res


def kernel(**inputs):
    out, _ = _run(inputs, trace=False)
    return out
